# revision 34
# baseline (speedup 1.0000x reference)
"""Causal linear attention (elu+1 feature map) for Trainium2, 8-core SPMD.

Sharding: core c = (a, b) with a = c//4 (batch of N=2... batch index a covers
batches {0,1} with 4 cores each) and b = c%4 (head quarter: heads [4b:4b+4)
of 16, i.e. feature columns [256b:256b+256) of the 1024-wide head dim).

Each core:
  P1: projects its batch's inputs through its head-quarter of Wq/Wk/Wv
      (bf16 matmuls, fp32 PSUM accumulation), applying phi(x)=elu(x)+1.
  P2: chunked causal linear attention for its 4 (head, batch) pairs
      (chunk = 128 sequence positions; intra-chunk masked A' matmuls +
      inter-chunk running state S).
  A2A: 8-way AllToAll redistributes attention outputs from
      (head-quarter, full batch-seq) sharding to (l-chunk, all heads).
  P3: output projection with Wo for its 256 output rows.

Host side only slices/transposes/replicates numpy inputs (pure layout) and
reassembles the sharded outputs.
"""

import sys

sys.path.insert(0, "/opt/trn_rl_repo")

import numpy as np

import concourse.bass as bass
import concourse.mybir as mybir
from concourse.tile import TileContext
from concourse.bass_utils import run_bass_kernel_spmd
from concourse.masks import make_upper_triangular

F32 = mybir.dt.float32
BF16 = mybir.dt.bfloat16

L = 1024          # sequence length
NB = 2            # batch
E = 1024          # embed dim
H = 16            # heads
D = 64            # head dim
EPS = 1e-6
N_CORES = 8
FPC = 256         # features per core (4 heads)
C = 128           # chunk size
NCH = L // C      # chunks per (head, batch) pair

LAST_RESULT = None  # set by kernel() for test harnesses


def _split_waits(nc, cap=1):
    """Walrus allows only one sync-wait on pseudo instructions (DMA triggers,
    collective triggers, drains). Move excess waits onto preceding single-wait
    NoOps on the same engine (engine FIFO order keeps semantics identical)."""
    ctr = 0
    for f in nc.m.functions:
        for blk in f.blocks:
            insts = list(blk.instructions)
            new = []
            changed = False
            for ins in insts:
                si = ins.sync_info
                waits = list(si.on_wait) if (si and si.on_wait) else []
                if len(waits) > cap:
                    excess, keep = waits[:-cap], waits[-cap:]
                    for w in excess:
                        new.append(mybir.InstNoOp(
                            name=f"wsplit-{ctr}",
                            sync_info=mybir.SyncInfo(on_wait=[w], on_update=[]),
                            engine=ins.engine,
                            bass_nofuse=True,
                        ))
                        ctr += 1
                    ins.sync_info = mybir.SyncInfo(on_wait=keep, on_update=si.on_update)
                    changed = True
                new.append(ins)
            if changed:
                blk.instructions = new
    return ctr


def _build(with_bias):
    nc = bass.Bass(num_devices=N_CORES)

    # Per-core external inputs (host feeds core-specific slices).
    xqT = nc.declare_dram_parameter("xqT", [E, L], BF16, isOutput=False)
    xkT = nc.declare_dram_parameter("xkT", [E, L], BF16, isOutput=False)
    xvT = nc.declare_dram_parameter("xvT", [E, L], BF16, isOutput=False)
    wqT = nc.declare_dram_parameter("wqT", [E, FPC], BF16, isOutput=False)
    wkT = nc.declare_dram_parameter("wkT", [E, FPC], BF16, isOutput=False)
    wvT = nc.declare_dram_parameter("wvT", [E, FPC], BF16, isOutput=False)
    woT = nc.declare_dram_parameter("woT", [E, E], BF16, isOutput=False)
    if with_bias:
        bq_d = nc.declare_dram_parameter("bq", [FPC, 1], F32, isOutput=False)
        bk_d = nc.declare_dram_parameter("bk", [FPC, 1], F32, isOutput=False)
        bv_d = nc.declare_dram_parameter("bv", [FPC, 1], F32, isOutput=False)
        bo_d = nc.declare_dram_parameter("bo", [1, E], F32, isOutput=False)
    out_d = nc.declare_dram_parameter("out", [NB, C, E], F32, isOutput=True)
    # token passthrough: lets a timing harness chain executions serially
    tok_i = nc.declare_dram_parameter("tok", [1, 1], F32, isOutput=False)
    tok_o = nc.declare_dram_parameter("tok_out", [1, 1], F32, isOutput=True)

    with TileContext(nc) as tc:
        with (
            tc.tile_pool(name="const", bufs=1) as constp,
            tc.tile_pool(name="xT", bufs=1) as xTp,
            tc.tile_pool(name="wT", bufs=1) as wTp,
            tc.tile_pool(name="woTp", bufs=1) as woTp,
            tc.tile_pool(name="proj", bufs=1) as projp,
            tc.tile_pool(name="tmp", bufs=3) as tmpp,
            tc.tile_pool(name="p2", bufs=6) as p2p
            ,tc.tile_pool(name="atT", bufs=1) as atTp,
            tc.tile_pool(name="attn", bufs=1) as attnp,
            tc.tile_pool(name="outp", bufs=1) as outp,
            tc.tile_pool(name="dram", bufs=1, space="DRAM") as dram,
            tc.tile_pool(name="ps", bufs=8, space="PSUM") as psp,
        ):
            # ---------------- constants ----------------
            mask2 = constp.tile([C, 4 * C], F32)      # 4x upper-tri mask
            for _h in range(4):
                make_upper_triangular(nc, mask2[:, _h * C:(_h + 1) * C],
                                      val=1.0, diag=True)

            if with_bias:
                bqs = [constp.tile([C, 1], F32, tag=f"bq{m}", name=f"bqs{m}") for m in range(2)]
                bks = [constp.tile([C, 1], F32, tag=f"bk{m}", name=f"bks{m}") for m in range(2)]
                bq1 = [constp.tile([C, 1], F32, tag=f"bq1{m}", name=f"bq1{m}") for m in range(2)]
                bk1 = [constp.tile([C, 1], F32, tag=f"bk1{m}", name=f"bk1{m}") for m in range(2)]
                for m in range(2):
                    nc.sync.dma_start(out=bqs[m][:], in_=bq_d[m * C:(m + 1) * C])
                    nc.sync.dma_start(out=bks[m][:], in_=bk_d[m * C:(m + 1) * C])
                    nc.vector.tensor_scalar(bq1[m][:], bqs[m][:], 1.0, None,
                                            op0=mybir.AluOpType.add)
                    nc.vector.tensor_scalar(bk1[m][:], bks[m][:], 1.0, None,
                                            op0=mybir.AluOpType.add)
                bo_row = constp.tile([1, E], F32)
                nc.sync.dma_start(out=bo_row[:], in_=bo_d[:])
                bkr = constp.tile([1, FPC], F32)
                bvr = constp.tile([1, FPC], F32)
                nc.sync.dma_start(out=bkr[:], in_=bk_d[:].rearrange("f one -> one f"))
                nc.sync.dma_start(out=bvr[:], in_=bv_d[:].rearrange("f one -> one f"))
                ones_row1 = constp.tile([1, C], F32)
                nc.vector.memset(ones_row1[:], 1.0)
                # broadcast-across-partitions tiles via K=1 matmuls
                bo_bc = constp.tile([C, E], F32)
                for nbk in range(2):
                    bo_ps = psp.tile([C, 512], F32, tag="ps")
                    nc.tensor.matmul(bo_ps[:, 0:512], ones_row1[:, 0:C],
                                     bo_row[:, nbk * 512:(nbk + 1) * 512],
                                     start=True, stop=True)
                    nc.vector.tensor_copy(bo_bc[:, nbk * 512:(nbk + 1) * 512], bo_ps[:])
                bk_bc = constp.tile([C, FPC], F32)
                bv_bc = constp.tile([C, FPC], F32)
                for row, bc in ((bkr, bk_bc), (bvr, bv_bc)):
                    bps = psp.tile([C, FPC], F32, tag="ps")
                    nc.tensor.matmul(bps[:], ones_row1[:, 0:C], row[:],
                                     start=True, stop=True)
                    nc.vector.tensor_copy(bc[:], bps[:])

            # ---------------- input loads (cast fp32 -> bf16) ----------------
            # HWDGE queues (SP/ACT) so no engine is blocked for the transfer;
            # issue order = DMA_ENGINES service order, so order by first use.
            def big_load(pool, src, kd, fd, nm, halves=1, eng=None):
                t = pool.tile([C, kd, fd], BF16, tag=nm, name=nm)
                src_r = src[:].rearrange("(k p) f -> p k f", p=C)
                step = kd // halves
                for h in range(halves):
                    ks = slice(h * step, (h + 1) * step)
                    e = eng if eng is not None else (nc.sync if h % 2 == 0
                                                     else nc.scalar)
                    e.dma_start(out=t[:, ks, :], in_=src_r[:, ks, :])
                return t
            wk_all = big_load(wTp, wkT, 8, FPC, "wk_all", halves=2)
            wv_all = big_load(wTp, wvT, 8, FPC, "wv_all", halves=2)
            wq_all = big_load(wTp, wqT, 8, FPC, "wq_all", halves=2)
            # serial xk -> xv -> xq: each projection needs its x in full, so
            # finish one tensor at a time; PE trickles k-major behind the feed
            # and only q's (short) chain runs after the feed drains.
            xk_all = xTp.tile([C, 8, L], BF16, tag="xk_all", name="xk_all")
            xv_all = xTp.tile([C, 8, L], BF16, tag="xv_all", name="xv_all")
            xq_all = xTp.tile([C, 8, L], BF16, tag="xq_all", name="xq_all")
            for i, (t, src) in enumerate(((xk_all, xkT), (xv_all, xvT),
                                          (xq_all, xqT))):
                for h in range(4):
                    ks = slice(h * 2, (h + 1) * 2)
                    e = nc.sync if (i * 4 + h) % 2 == 0 else nc.scalar
                    e.dma_start(
                        out=t[:, ks, :],
                        in_=src[:].rearrange("(k p) f -> p k f", p=C)[:, ks, :])
            xv_sb = [xv_all[:, k, :] for k in range(8)]
            xk_sb = [xk_all[:, k, :] for k in range(8)]
            xq_sb = [xq_all[:, k, :] for k in range(8)]
            wv_sb = [wv_all[:, k, :] for k in range(8)]
            wk_sb = [wk_all[:, k, :] for k in range(8)]
            wq_sb = [wq_all[:, k, :] for k in range(8)]

            # ---------------- P1a: natural-layout k_nat / v_nat ----------------
            # k-slice-major accumulation with one PSUM bank per l-chain (a
            # bank holds at most one open accumulation group), so PE consumes
            # each arriving x k-slice immediately.
            kpsl = [psp.tile([C, FPC], F32, tag="ps", name=f"kps{lt}")
                    for lt in range(8)]
            for k in range(8):
                for lt in range(8):
                    nc.tensor.matmul(kpsl[lt][:],
                                     xk_sb[k][:, lt * C:(lt + 1) * C],
                                     wk_sb[k][:],
                                     start=(k == 0), stop=(k == 7))
            knat_all = projp.tile([C, 8, FPC], BF16, tag="knat",
                                  name="knat_all")
            for lt in range(8):
                psk = kpsl[lt]
                if with_bias:
                    psb = tmpp.tile([C, FPC], F32, tag="psb")
                    nc.vector.tensor_add(psb[:], psk[:], bk_bc[:])
                    psk = psb
                # phi(x) = max(exp(min(x,0)), x+1); single PSUM read:
                # u = x+1, relu(-x) = relu(1-u), exp(min(x,0)) = exp(-relu(-x))
                tu = tmpp.tile([C, FPC], F32, tag="tu")
                nc.vector.tensor_scalar(tu[:], psk[:], 1.0, None,
                                        op0=mybir.AluOpType.add)
                tmin = tmpp.tile([C, FPC], F32, tag="tmin")
                nc.scalar.activation(tmin[:], tu[:],
                                     mybir.ActivationFunctionType.Relu,
                                     scale=-1.0, bias=1.0)
                texp = tmpp.tile([C, FPC], F32, tag="texp")
                nc.scalar.activation(texp[:], tmin[:],
                                     mybir.ActivationFunctionType.Exp,
                                     scale=-1.0)
                nc.vector.tensor_max(knat_all[:, lt, :], texp[:], tu[:])

            def knat_ap(c, cols):
                return knat_all[:, c, cols]

            vpsl = [psp.tile([C, FPC], F32, tag="ps", name=f"vps{lt}")
                    for lt in range(8)]
            for k in range(8):
                for lt in range(8):
                    nc.tensor.matmul(vpsl[lt][:],
                                     xv_sb[k][:, lt * C:(lt + 1) * C],
                                     wv_sb[k][:],
                                     start=(k == 0), stop=(k == 7))
            vnat = []
            for lt in range(8):
                vt = projp.tile([C, 4 * 65], BF16, tag=f"vn{lt}")
                nc.gpsimd.memset(vt[:], 1.0)   # ones columns (and init)
                vna = vt[:].rearrange("p (f c) -> p f c", f=4)
                vsrc = vpsl[lt][:].rearrange("p (f c) -> p f c", f=4)
                if with_bias:
                    nc.vector.tensor_add(vna[:, :, 0:64], vsrc,
                                         bv_bc[:].rearrange("p (f c) -> p f c",
                                                            f=4))
                else:
                    nc.scalar.copy(vna[:, :, 0:64], vsrc)
                vnat.append(vt)

            # ---------------- P1b: feature-major q' ----------------
            # per-pair tiles (64, L) bf16, base partition 0
            qp_t = [projp.tile([D, L], BF16, tag=f"qp{p}", name=f"qp{p}") for p in range(4)]
            for m in range(2):
                for nbk in range(2):
                    ps = psp.tile([C, 512], F32, tag="ps")
                    for k in range(8):
                        nc.tensor.matmul(
                            ps[:], wq_sb[k][:, m * C:(m + 1) * C],
                            xq_sb[k][:, nbk * 512:(nbk + 1) * 512],
                            start=(k == 0), stop=(k == 7))
                    cs = slice(nbk * 512, (nbk + 1) * 512)
                    tu = tmpp.tile([C, 512], F32, tag="tu")
                    if with_bias:
                        nc.vector.tensor_scalar(tu[:], ps[:], bq1[m][:], None,
                                                op0=mybir.AluOpType.add)
                    else:
                        nc.vector.tensor_scalar(tu[:], ps[:], 1.0, None,
                                                op0=mybir.AluOpType.add)
                    tmin = tmpp.tile([C, 512], F32, tag="tmin")
                    nc.scalar.activation(tmin[:], tu[:],
                                         mybir.ActivationFunctionType.Relu,
                                         scale=-1.0, bias=1.0)
                    texp = tmpp.tile([C, 512], F32, tag="texp")
                    nc.scalar.activation(texp[:], tmin[:],
                                         mybir.ActivationFunctionType.Exp,
                                         scale=-1.0)
                    for half in range(2):
                        pr = slice(half * D, (half + 1) * D)
                        nc.vector.tensor_max(qp_t[2 * m + half][:, cs],
                                             texp[pr, 0:512], tu[pr, 0:512])

            # ---------------- feature-major k' via DMA transpose ----------
            # two batched block-transposes: kp2x[:, t, :] is the transpose of
            # knat block t = 2*lt + hh (hh = pair-half); odd pairs get a
            # base-partition-0 copy (matmul needs equal base partitions)
            kp2x = projp.tile([C, 16, C], BF16, tag="kp2x", name="kp2x")
            kp_odd = projp.tile([D, 16, C], BF16, tag="kpo", name="kpo")
            for lh in range(2):
                nc.scalar.dma_start(
                    out=kp2x[:, 8 * lh:8 * (lh + 1), :],
                    in_=knat_all[:, 4 * lh:4 * (lh + 1), :],
                    transpose=True)
                nc.vector.tensor_copy(kp_odd[:, 8 * lh:8 * (lh + 1), :],
                                      kp2x[D:C, 8 * lh:8 * (lh + 1), :])

            def kp_ap(p, c):
                t = 2 * c + p // 2
                if p % 2 == 0:
                    return kp2x[0:D, t, :]
                return kp_odd[:, t, :]

            # ---------------- P2: chunked causal linear attention ----------------
            # natural-layout O'^T per chunk: [128 t, 65] = A'_c^T-contracted
            # V^ plus prefix-state terms; division is a per-partition scalar
            # multiply straight out of PSUM (denominator = col 64).
            # attn_nat[lt]: [128 l, 256 f] (all 4 pairs of l-chunk lt).
            attn_nat = p2p.tile([C, 8, FPC], BF16, tag="anat",
                                name="attn_nat", bufs=1)
            for p in range(4):
                vcols = slice(p * 65, p * 65 + 65)
                kcols = slice(p * D, (p + 1) * D)
                # per-chunk states T_c = K_c^T V^_c: 8 independent matmuls
                # into 2 PSUM tiles, one ACT copy each, then 4 tiny bf16 adds
                # for the prefix cover (no PSUM<->ACT ping-pong chain).
                tsb = p2p.tile([D, 8, 65], BF16, tag="ts", bufs=4,
                               name=f"ts_{p}")
                for half in range(2):
                    ups = psp.tile([D, 4, 65], F32, tag="ps",
                                   name=f"ups{p}_{half}")
                    for c4 in range(4):
                        c = 4 * half + c4
                        nc.tensor.matmul(ups[:, c4, :], knat_ap(c, kcols),
                                         vnat[c][:, vcols],
                                         start=True, stop=True)
                    nc.scalar.copy(tsb[:, 4 * half:4 * half + 4, :], ups[:])
                usb = p2p.tile([D, 4, 65], BF16, tag="us", bufs=4,
                               name=f"us_{p}")
                # usb: 0=u01, 1=t23, 2=u03, 3=u45
                nc.vector.tensor_add(usb[:, 0, :], tsb[:, 0, :], tsb[:, 1, :])
                nc.vector.tensor_add(usb[:, 1, :], tsb[:, 2, :], tsb[:, 3, :])
                nc.vector.tensor_add(usb[:, 2, :], usb[:, 0, :], usb[:, 1, :])
                nc.vector.tensor_add(usb[:, 3, :], tsb[:, 4, :], tsb[:, 5, :])
                t0, t2, t4, t6 = (tsb[:, 0, :], tsb[:, 2, :],
                                  tsb[:, 4, :], tsb[:, 6, :])
                u01, t23, u03, u45 = (usb[:, 0, :], usb[:, 1, :],
                                      usb[:, 2, :], usb[:, 3, :])
                # minimal prefix cover of T_0..T_{c-1} for each chunk c
                terms = {
                    0: [], 1: [t0], 2: [u01], 3: [u01, t2],
                    4: [u03], 5: [u03, t4], 6: [u03, u45],
                    7: [u03, u45, t6],
                }
                NB_CH = 4
                for cp2 in range(NCH // NB_CH):
                    cI0 = NB_CH * cp2
                    # A'[s,t] for NB_CH chunks side by side (causal mask s<=t)
                    aps = psp.tile([C, NB_CH * C], F32, tag="ps")
                    for h in range(NB_CH):
                        cs = slice((cI0 + h) * C, (cI0 + h + 1) * C)
                        nc.tensor.matmul(aps[:, h * C:(h + 1) * C],
                                         kp_ap(p, cI0 + h), qp_t[p][:, cs],
                                         start=True, stop=True)
                    asb = p2p.tile([C, NB_CH * C], BF16, tag="asb")
                    nc.vector.tensor_mul(asb[:], aps[:], mask2[:])
                    # O'^T (128t, 65) = A'_c^T-contraction of V^ + Q'^T S
                    ops = psp.tile([C, NB_CH, 65], F32, tag="ps")
                    for h in range(NB_CH):
                        cI = cI0 + h
                        cs = slice(cI * C, (cI + 1) * C)
                        tl = terms[cI]
                        nc.tensor.matmul(ops[:, h, :], asb[:, h * C:(h + 1) * C],
                                         vnat[cI][:, vcols],
                                         start=True, stop=not tl)
                        for i2, tt in enumerate(tl):
                            nc.tensor.matmul(ops[:, h, :], qp_t[p][:, cs],
                                             tt,
                                             start=False, stop=(i2 == len(tl) - 1))
                    # division: denom = q.z is strictly positive and >=O(10)
                    # at these input scales; the reference's +1e-6 contributes
                    # ~1e-8 relative, far below bf16 precision, so skip it.
                    rcp = p2p.tile([C, NB_CH], F32, tag="rcp", bufs=4)
                    nc.vector.reciprocal(rcp[:], ops[:, :, 64])
                    for h in range(NB_CH):
                        dst = attn_nat[:, cI0 + h, p * D:(p + 1) * D]
                        if h % 2 == 0:
                            nc.vector.tensor_scalar(
                                dst, ops[:, h, 0:64], rcp[:, h:h + 1], None,
                                op0=mybir.AluOpType.mult)
                        else:
                            nc.scalar.activation(
                                dst, ops[:, h, 0:64],
                                mybir.ActivationFunctionType.Copy,
                                scale=rcp[:, h:h + 1])

            # ---------------- attn natural -> feature-major (DMA transpose) --
            # att3[:, t, :] = transpose of attn_nat block t = 2*lt + h
            # (h = pair-half), via two batched block-transposes
            att3 = attnp.tile([C, 16, C], BF16, tag="att3")
            for lh in range(2):
                nc.sync.dma_start(
                    out=att3[:, 8 * lh:8 * (lh + 1), :],
                    in_=attn_nat[:, 4 * lh:4 * (lh + 1), :],
                    transpose=True)

            # ---------------- A2A: redistribute attention outputs ----------------
            # single merged collective: buffer row r = j*256 + h*128 + f
            # (j = dest l-chunk, h = pair-half, f = feature-in-half); shard j
            # is the contiguous 256-row block j.
            a2a_in = dram.tile([N_CORES, 2 * C, C], BF16, tag="a2ain")
            a2a_out = dram.tile([N_CORES, 2 * C, C], BF16, tag="a2aout")
            # stage in halves (l-chunks 0-3 / 4-7, both pair-halves at once)
            for lq in range(2):
                nc.sync.dma_start(
                    out=a2a_in[lq * 4:(lq + 1) * 4, :, :]
                        .rearrange("j (h f) l -> f (j h) l", h=2),
                    in_=att3[:, 8 * lq:8 * (lq + 1), :],
                )
            # 2D APs: the cost model prices a collective by its non-leading
            # dims, so present the buffer as [2048, 128]
            nc.gpsimd.collective_compute(
                "AllToAll",
                mybir.AluOpType.bypass,
                replica_groups=[list(range(N_CORES))],
                ins=[a2a_in[:].rearrange("j f l -> (j f) l")],
                outs=[a2a_out[:].rearrange("j f l -> (j f) l")],
            )

            # ---------------- woT load (single big cast-DMA) ----------------
            wo_all = big_load(woTp, woT, 8, E, "wo_all", halves=2)
            wo_sb = [wo_all[:, k, :] for k in range(8)]

            # ---------------- P3: output projection ----------------
            # a2a_out rows for batch n are (b h f)-ordered: block s = 2b+h
            # matches wo_sb[s] (feature rows [128s, 128s+128)). Two DMAs per
            # batch so the kk-accumulation can start on the first half.
            ath = {n: atTp.tile([C, 8, C], BF16, tag=f"ath{n}", name=f"ath{n}")
                   for n in range(NB)}
            for hv in range(2):
                for n in range(NB):
                    e = nc.sync if n == 0 else nc.scalar
                    e.dma_start(
                        out=ath[n][:, hv * 4:(hv + 1) * 4, :],
                        in_=a2a_out[4 * n + 2 * hv:4 * n + 2 * hv + 2, :, :]
                            .rearrange("b (h f) l -> f (b h) l", f=C))
            ps3 = {}
            for n in range(NB):
                for nbk in range(2):
                    ps3[(n, nbk)] = psp.tile([C, 512], F32, tag="ps",
                                             name=f"ps3_{n}_{nbk}")
            # hv-outer: all four PSUM groups start on the first ath arrivals
            for hv in range(2):
                for n in range(NB):
                    for nbk in range(2):
                        for s4 in range(4):
                            kk = 4 * hv + s4
                            nc.tensor.matmul(
                                ps3[(n, nbk)][:],
                                ath[n][:, kk, :],
                                wo_sb[kk][:, nbk * 512:(nbk + 1) * 512],
                                start=(hv == 0 and s4 == 0),
                                stop=(hv == 1 and s4 == 3))
            for n in range(NB):
                osb = outp.tile([C, E], F32, tag=f"osb{n}", name=f"osb{n}")
                for nbk in range(2):
                    cs = slice(nbk * 512, (nbk + 1) * 512)
                    if with_bias:
                        nc.vector.tensor_add(osb[:, cs], ps3[(n, nbk)][:],
                                             bo_bc[:, cs])
                    elif (2 * n + nbk) % 2 == 0:
                        nc.scalar.copy(osb[:, cs], ps3[(n, nbk)][:])
                    else:
                        nc.vector.tensor_copy(osb[:, cs], ps3[(n, nbk)][:])
                    e = nc.sync if n == 0 else nc.scalar
                    e.dma_start(out=out_d[n][:, cs], in_=osb[:, cs])
            nc.sync.dma_start(out=tok_o[:], in_=tok_i[:])

    _split_waits(nc)
    return nc


def _run_pjrt_timed(nc, in_maps, time_iters=0):
    """Replicates bass2jax.run_bass_via_pjrt's multi-core path, but keeps
    inputs device-resident and (optionally) times repeated executions.
    Returns (results, best_exec_seconds_or_None)."""
    import time as _time
    import jax
    from jax.sharding import Mesh, PartitionSpec, NamedSharding
    from jax.experimental.shard_map import shard_map
    from concourse import bass2jax, mybir as mb

    bass2jax.install_neuronx_cc_hook()
    n_cores = len(in_maps)
    partition_name = nc.partition_id_tensor.name if nc.partition_id_tensor else None

    in_names, out_names, out_avals, zero_outs = [], [], [], []
    for alloc in nc.m.functions[0].allocations:
        if not isinstance(alloc, mb.MemoryLocationSet):
            continue
        name = alloc.memorylocations[0].name
        if alloc.kind == "ExternalInput":
            if name != partition_name:
                in_names.append(name)
        elif alloc.kind == "ExternalOutput":
            out_names.append(name)
            shape = tuple(alloc.tensor_shape)
            dtype = mb.dt.np(alloc.dtype)
            out_avals.append(jax.core.ShapedArray(shape, dtype))
            zero_outs.append(np.zeros(shape, dtype))
    n_params = len(in_names)
    in_names.extend(out_names)
    if partition_name is not None:
        in_names.append(partition_name)

    chain = int(__import__("os").environ.get("TRN_KERNEL_CHAIN", "1"))
    tok_in_idx = in_names.index("tok") if "tok" in in_names else None
    tok_out_idx = out_names.index("tok_out") if "tok_out" in out_names else None

    def _body(*args):
        operands = list(args)
        pid = bass2jax.partition_id_tensor() if partition_name is not None else None
        outs = None
        for _ in range(chain):
            ops = list(operands)
            if outs is not None and tok_in_idx is not None:
                ops[tok_in_idx] = outs[tok_out_idx]  # serialize iterations
            if pid is not None:
                ops.append(pid)
            outs = bass2jax._bass_exec_p.bind(
                *ops,
                out_avals=tuple(out_avals),
                in_names=tuple(in_names),
                out_names=tuple(out_names),
                lowering_input_output_aliases=(),
                sim_require_finite=True,
                sim_require_nnan=True,
                nc=nc,
            )
        return tuple(outs)

    devices = jax.devices()[:n_cores]
    mesh = Mesh(np.asarray(devices), ("core",))
    in_specs = (PartitionSpec("core"),) * (n_params + len(out_names))
    out_specs = (PartitionSpec("core"),) * len(out_names)
    sharded = jax.jit(
        shard_map(_body, mesh=mesh, in_specs=in_specs, out_specs=out_specs,
                  check_rep=False),
        keep_unused=True,
    )
    per_core = [[np.asarray(m[name]) for name in in_names[:n_params]]
                for m in in_maps]
    concat_in = [np.concatenate([per_core[c][i] for c in range(n_cores)], axis=0)
                 for i in range(n_params)]
    concat_zeros = [np.zeros((n_cores * z.shape[0], *z.shape[1:]), z.dtype)
                    for z in zero_outs]
    shd = NamedSharding(mesh, PartitionSpec("core"))
    dev_in = [jax.device_put(a, shd) for a in concat_in + concat_zeros]

    out_arrs = sharded(*dev_in)
    jax.block_until_ready(out_arrs)
    best = None
    for _ in range(time_iters):
        t0 = _time.perf_counter()
        out_arrs2 = sharded(*dev_in)
        jax.block_until_ready(out_arrs2)
        dt = _time.perf_counter() - t0
        best = dt if best is None or dt < best else best
    results = [
        {name: np.asarray(out_arrs[i]).reshape(n_cores, *out_avals[i].shape)[c]
         for i, name in enumerate(out_names)}
        for c in range(n_cores)
    ]
    return results, best


def kernel(**inputs):
    global LAST_RESULT
    import os

    query = np.asarray(inputs["query"], np.float32)
    key = np.asarray(inputs["key"], np.float32)
    value = np.asarray(inputs["value"], np.float32)
    Wq = np.asarray(inputs["Wq"], np.float32)
    Wk = np.asarray(inputs["Wk"], np.float32)
    Wv = np.asarray(inputs["Wv"], np.float32)
    Wo = np.asarray(inputs["Wo"], np.float32)
    bq = np.asarray(inputs["bq"], np.float32)
    bk = np.asarray(inputs["bk"], np.float32)
    bv = np.asarray(inputs["bv"], np.float32)
    bo = np.asarray(inputs["bo"], np.float32)

    with_bias = any(np.any(b) for b in (bq, bk, bv, bo))
    nc = _build(with_bias)

    from ml_dtypes import bfloat16 as _bf16

    def _b(x):
        return np.ascontiguousarray(x).astype(_bf16)

    woT_full = _b(Wo.T)
    in_maps = []
    for c in range(N_CORES):
        a, b = c // 4, c % 4
        F = slice(FPC * b, FPC * (b + 1))
        m = {
            "xqT": _b(query[:, a, :].T),
            "xkT": _b(key[:, a, :].T),
            "xvT": _b(value[:, a, :].T),
            "wqT": _b(Wq[F, :].T),
            "wkT": _b(Wk[F, :].T),
            "wvT": _b(Wv[F, :].T),
            "woT": woT_full,
            "tok": np.zeros((1, 1), np.float32),
        }
        if with_bias:
            m["bq"] = np.ascontiguousarray(bq[F].reshape(FPC, 1))
            m["bk"] = np.ascontiguousarray(bk[F].reshape(FPC, 1))
            m["bv"] = np.ascontiguousarray(bv[F].reshape(FPC, 1))
            m["bo"] = np.ascontiguousarray(bo.reshape(1, E))
        in_maps.append(m)

    time_iters = int(os.environ.get("TRN_KERNEL_TIME_ITERS", "0"))
    results, best = _run_pjrt_timed(nc, in_maps, time_iters=time_iters)
    LAST_RESULT = {"results": results, "best_exec_s": best}

    out = np.empty((L, NB, E), np.float32)
    for c in range(N_CORES):
        o = results[c]["out"]  # (NB, C, E): my l-chunk rows for both batches
        for n in range(NB):
            out[c * C:(c + 1) * C, n, :] = o[n]
    return out



# revision 35
# speedup vs baseline: 1.0616x; 1.0616x over previous
"""Causal linear attention (elu+1 feature map) for Trainium2, 8-core SPMD.

Sharding: core c = (a, b) with a = c//4 (batch of N=2... batch index a covers
batches {0,1} with 4 cores each) and b = c%4 (head quarter: heads [4b:4b+4)
of 16, i.e. feature columns [256b:256b+256) of the 1024-wide head dim).

Each core:
  P1: projects its batch's inputs through its head-quarter of Wq/Wk/Wv
      (bf16 matmuls, fp32 PSUM accumulation), applying phi(x)=elu(x)+1.
  P2: chunked causal linear attention for its 4 (head, batch) pairs
      (chunk = 128 sequence positions; intra-chunk masked A' matmuls +
      inter-chunk running state S).
  A2A: 8-way AllToAll redistributes attention outputs from
      (head-quarter, full batch-seq) sharding to (l-chunk, all heads).
  P3: output projection with Wo for its 256 output rows.

Host side only slices/transposes/replicates numpy inputs (pure layout) and
reassembles the sharded outputs.
"""

import sys

sys.path.insert(0, "/opt/trn_rl_repo")

import numpy as np

import concourse.bass as bass
import concourse.mybir as mybir
from concourse.tile import TileContext
from concourse.bass_utils import run_bass_kernel_spmd
from concourse.masks import make_upper_triangular

F32 = mybir.dt.float32
BF16 = mybir.dt.bfloat16

L = 1024          # sequence length
NB = 2            # batch
E = 1024          # embed dim
H = 16            # heads
D = 64            # head dim
EPS = 1e-6
N_CORES = 8
FPC = 256         # features per core (4 heads)
C = 128           # chunk size
NCH = L // C      # chunks per (head, batch) pair

LAST_RESULT = None  # set by kernel() for test harnesses


def _split_waits(nc, cap=1):
    """Walrus allows only one sync-wait on pseudo instructions (DMA triggers,
    collective triggers, drains). Move excess waits onto preceding single-wait
    NoOps on the same engine (engine FIFO order keeps semantics identical)."""
    ctr = 0
    for f in nc.m.functions:
        for blk in f.blocks:
            insts = list(blk.instructions)
            new = []
            changed = False
            for ins in insts:
                si = ins.sync_info
                waits = list(si.on_wait) if (si and si.on_wait) else []
                if len(waits) > cap:
                    excess, keep = waits[:-cap], waits[-cap:]
                    for w in excess:
                        new.append(mybir.InstNoOp(
                            name=f"wsplit-{ctr}",
                            sync_info=mybir.SyncInfo(on_wait=[w], on_update=[]),
                            engine=ins.engine,
                            bass_nofuse=True,
                        ))
                        ctr += 1
                    ins.sync_info = mybir.SyncInfo(on_wait=keep, on_update=si.on_update)
                    changed = True
                new.append(ins)
            if changed:
                blk.instructions = new
    return ctr


def _build(with_bias):
    nc = bass.Bass(num_devices=N_CORES)

    # Per-core external inputs (host feeds core-specific slices).
    xqT = nc.declare_dram_parameter("xqT", [E, L], BF16, isOutput=False)
    xkT = nc.declare_dram_parameter("xkT", [E, L], BF16, isOutput=False)
    xvT = nc.declare_dram_parameter("xvT", [E, L], BF16, isOutput=False)
    wqT = nc.declare_dram_parameter("wqT", [E, FPC], BF16, isOutput=False)
    wkT = nc.declare_dram_parameter("wkT", [E, FPC], BF16, isOutput=False)
    wvT = nc.declare_dram_parameter("wvT", [E, FPC], BF16, isOutput=False)
    woT = nc.declare_dram_parameter("woT", [E, E], BF16, isOutput=False)
    if with_bias:
        bq_d = nc.declare_dram_parameter("bq", [FPC, 1], F32, isOutput=False)
        bk_d = nc.declare_dram_parameter("bk", [FPC, 1], F32, isOutput=False)
        bv_d = nc.declare_dram_parameter("bv", [FPC, 1], F32, isOutput=False)
        bo_d = nc.declare_dram_parameter("bo", [1, E], F32, isOutput=False)
    out_d = nc.declare_dram_parameter("out", [NB, C, E], F32, isOutput=True)
    # token passthrough: lets a timing harness chain executions serially
    tok_i = nc.declare_dram_parameter("tok", [1, 1], F32, isOutput=False)
    tok_o = nc.declare_dram_parameter("tok_out", [1, 1], F32, isOutput=True)

    with TileContext(nc) as tc:
        with (
            tc.tile_pool(name="const", bufs=1) as constp,
            tc.tile_pool(name="xT", bufs=1) as xTp,
            tc.tile_pool(name="wT", bufs=1) as wTp,
            tc.tile_pool(name="woTp", bufs=1) as woTp,
            tc.tile_pool(name="proj", bufs=1) as projp,
            tc.tile_pool(name="tmp", bufs=3) as tmpp,
            tc.tile_pool(name="p2", bufs=6) as p2p
            ,tc.tile_pool(name="atT", bufs=1) as atTp,
            tc.tile_pool(name="attn", bufs=1) as attnp,
            tc.tile_pool(name="outp", bufs=1) as outp,
            tc.tile_pool(name="dram", bufs=1, space="DRAM") as dram,
            tc.tile_pool(name="ps", bufs=8, space="PSUM") as psp,
        ):
            # ---------------- constants ----------------
            mask2 = constp.tile([C, 4 * C], F32)      # 4x upper-tri mask
            for _h in range(4):
                make_upper_triangular(nc, mask2[:, _h * C:(_h + 1) * C],
                                      val=1.0, diag=True)

            if with_bias:
                bqs = [constp.tile([C, 1], F32, tag=f"bq{m}", name=f"bqs{m}") for m in range(2)]
                bks = [constp.tile([C, 1], F32, tag=f"bk{m}", name=f"bks{m}") for m in range(2)]
                bq1 = [constp.tile([C, 1], F32, tag=f"bq1{m}", name=f"bq1{m}") for m in range(2)]
                bk1 = [constp.tile([C, 1], F32, tag=f"bk1{m}", name=f"bk1{m}") for m in range(2)]
                for m in range(2):
                    nc.sync.dma_start(out=bqs[m][:], in_=bq_d[m * C:(m + 1) * C])
                    nc.sync.dma_start(out=bks[m][:], in_=bk_d[m * C:(m + 1) * C])
                    nc.vector.tensor_scalar(bq1[m][:], bqs[m][:], 1.0, None,
                                            op0=mybir.AluOpType.add)
                    nc.vector.tensor_scalar(bk1[m][:], bks[m][:], 1.0, None,
                                            op0=mybir.AluOpType.add)
                bo_row = constp.tile([1, E], F32)
                nc.sync.dma_start(out=bo_row[:], in_=bo_d[:])
                bkr = constp.tile([1, FPC], F32)
                bvr = constp.tile([1, FPC], F32)
                nc.sync.dma_start(out=bkr[:], in_=bk_d[:].rearrange("f one -> one f"))
                nc.sync.dma_start(out=bvr[:], in_=bv_d[:].rearrange("f one -> one f"))
                ones_row1 = constp.tile([1, C], F32)
                nc.vector.memset(ones_row1[:], 1.0)
                # broadcast-across-partitions tiles via K=1 matmuls
                bo_bc = constp.tile([C, E], F32)
                for nbk in range(2):
                    bo_ps = psp.tile([C, 512], F32, tag="ps")
                    nc.tensor.matmul(bo_ps[:, 0:512], ones_row1[:, 0:C],
                                     bo_row[:, nbk * 512:(nbk + 1) * 512],
                                     start=True, stop=True)
                    nc.vector.tensor_copy(bo_bc[:, nbk * 512:(nbk + 1) * 512], bo_ps[:])
                bk_bc = constp.tile([C, FPC], F32)
                bv_bc = constp.tile([C, FPC], F32)
                for row, bc in ((bkr, bk_bc), (bvr, bv_bc)):
                    bps = psp.tile([C, FPC], F32, tag="ps")
                    nc.tensor.matmul(bps[:], ones_row1[:, 0:C], row[:],
                                     start=True, stop=True)
                    nc.vector.tensor_copy(bc[:], bps[:])

            # ---------------- input loads (cast fp32 -> bf16) ----------------
            # HWDGE queues (SP/ACT) so no engine is blocked for the transfer;
            # issue order = DMA_ENGINES service order, so order by first use.
            def big_load(pool, src, kd, fd, nm, halves=1, eng=None):
                t = pool.tile([C, kd, fd], BF16, tag=nm, name=nm)
                src_r = src[:].rearrange("(k p) f -> p k f", p=C)
                step = kd // halves
                for h in range(halves):
                    ks = slice(h * step, (h + 1) * step)
                    e = eng if eng is not None else (nc.sync if h % 2 == 0
                                                     else nc.scalar)
                    e.dma_start(out=t[:, ks, :], in_=src_r[:, ks, :])
                return t
            wk_all = big_load(wTp, wkT, 8, FPC, "wk_all", halves=2)
            wv_all = big_load(wTp, wvT, 8, FPC, "wv_all", halves=2)
            wq_all = big_load(wTp, wqT, 8, FPC, "wq_all", halves=2)
            # serial xk -> xv -> xq: each projection needs its x in full, so
            # finish one tensor at a time; PE trickles k-major behind the feed
            # and only q's (short) chain runs after the feed drains.
            xk_all = xTp.tile([C, 8, L], BF16, tag="xk_all", name="xk_all")
            xv_all = xTp.tile([C, 8, L], BF16, tag="xv_all", name="xv_all")
            xq_all = xTp.tile([C, 8, L], BF16, tag="xq_all", name="xq_all")
            for i, (t, src) in enumerate(((xk_all, xkT), (xv_all, xvT),
                                          (xq_all, xqT))):
                for h in range(4):
                    ks = slice(h * 2, (h + 1) * 2)
                    e = nc.sync if (i * 4 + h) % 2 == 0 else nc.scalar
                    e.dma_start(
                        out=t[:, ks, :],
                        in_=src[:].rearrange("(k p) f -> p k f", p=C)[:, ks, :])
            xv_sb = [xv_all[:, k, :] for k in range(8)]
            xk_sb = [xk_all[:, k, :] for k in range(8)]
            xq_sb = [xq_all[:, k, :] for k in range(8)]
            wv_sb = [wv_all[:, k, :] for k in range(8)]
            wk_sb = [wk_all[:, k, :] for k in range(8)]
            wq_sb = [wq_all[:, k, :] for k in range(8)]

            # ---------------- P1a: natural-layout k_nat / v_nat ----------------
            # k-slice-major accumulation with one PSUM bank per l-chain (a
            # bank holds at most one open accumulation group), so PE consumes
            # each arriving x k-slice immediately.
            kpsl = [psp.tile([C, FPC], F32, tag="ps", name=f"kps{lt}")
                    for lt in range(8)]
            for k in range(8):
                for lt in range(8):
                    nc.tensor.matmul(kpsl[lt][:],
                                     xk_sb[k][:, lt * C:(lt + 1) * C],
                                     wk_sb[k][:],
                                     start=(k == 0), stop=(k == 7))
            knat_all = projp.tile([C, 8, FPC], BF16, tag="knat",
                                  name="knat_all")
            for lt in range(8):
                psk = kpsl[lt]
                if with_bias:
                    psb = tmpp.tile([C, FPC], F32, tag="psb")
                    nc.vector.tensor_add(psb[:], psk[:], bk_bc[:])
                    psk = psb
                # phi(x) = max(exp(min(x,0)), x+1); single PSUM read:
                # u = x+1, relu(-x) = relu(1-u), exp(min(x,0)) = exp(-relu(-x))
                tu = tmpp.tile([C, FPC], BF16, tag="tu")
                nc.vector.tensor_scalar(tu[:], psk[:], 1.0, None,
                                        op0=mybir.AluOpType.add)
                tmin = tmpp.tile([C, FPC], BF16, tag="tmin")
                nc.scalar.activation(tmin[:], tu[:],
                                     mybir.ActivationFunctionType.Relu,
                                     scale=-1.0, bias=1.0)
                texp = tmpp.tile([C, FPC], BF16, tag="texp")
                nc.scalar.activation(texp[:], tmin[:],
                                     mybir.ActivationFunctionType.Exp,
                                     scale=-1.0)
                nc.vector.tensor_max(knat_all[:, lt, :], texp[:], tu[:])

            def knat_ap(c, cols):
                return knat_all[:, c, cols]

            vpsl = [psp.tile([C, FPC], F32, tag="ps", name=f"vps{lt}")
                    for lt in range(8)]
            for k in range(8):
                for lt in range(8):
                    nc.tensor.matmul(vpsl[lt][:],
                                     xv_sb[k][:, lt * C:(lt + 1) * C],
                                     wv_sb[k][:],
                                     start=(k == 0), stop=(k == 7))
            vnat = []
            for lt in range(8):
                vt = projp.tile([C, 4 * 65], BF16, tag=f"vn{lt}")
                nc.gpsimd.memset(vt[:], 1.0)   # ones columns (and init)
                vna = vt[:].rearrange("p (f c) -> p f c", f=4)
                vsrc = vpsl[lt][:].rearrange("p (f c) -> p f c", f=4)
                if with_bias:
                    nc.vector.tensor_add(vna[:, :, 0:64], vsrc,
                                         bv_bc[:].rearrange("p (f c) -> p f c",
                                                            f=4))
                else:
                    nc.scalar.copy(vna[:, :, 0:64], vsrc)
                vnat.append(vt)

            # ---------------- P1b: feature-major q' ----------------
            # per-pair tiles (64, L) bf16, base partition 0
            qp_t = [projp.tile([D, L], BF16, tag=f"qp{p}", name=f"qp{p}") for p in range(4)]
            for m in range(2):
                for nbk in range(2):
                    ps = psp.tile([C, 512], F32, tag="ps")
                    for k in range(8):
                        nc.tensor.matmul(
                            ps[:], wq_sb[k][:, m * C:(m + 1) * C],
                            xq_sb[k][:, nbk * 512:(nbk + 1) * 512],
                            start=(k == 0), stop=(k == 7))
                    cs = slice(nbk * 512, (nbk + 1) * 512)
                    tu = tmpp.tile([C, 512], BF16, tag="tuq")
                    if with_bias:
                        nc.vector.tensor_scalar(tu[:], ps[:], bq1[m][:], None,
                                                op0=mybir.AluOpType.add)
                    else:
                        nc.vector.tensor_scalar(tu[:], ps[:], 1.0, None,
                                                op0=mybir.AluOpType.add)
                    tmin = tmpp.tile([C, 512], BF16, tag="tminq")
                    nc.scalar.activation(tmin[:], tu[:],
                                         mybir.ActivationFunctionType.Relu,
                                         scale=-1.0, bias=1.0)
                    texp = tmpp.tile([C, 512], BF16, tag="texpq")
                    nc.scalar.activation(texp[:], tmin[:],
                                         mybir.ActivationFunctionType.Exp,
                                         scale=-1.0)
                    for half in range(2):
                        pr = slice(half * D, (half + 1) * D)
                        nc.vector.tensor_max(qp_t[2 * m + half][:, cs],
                                             texp[pr, 0:512], tu[pr, 0:512])

            # ---------------- feature-major k' via DMA transpose ----------
            # two batched block-transposes: kp2x[:, t, :] is the transpose of
            # knat block t = 2*lt + hh (hh = pair-half); odd pairs get a
            # base-partition-0 copy (matmul needs equal base partitions)
            kp2x = projp.tile([C, 16, C], BF16, tag="kp2x", name="kp2x")
            kp_odd = projp.tile([D, 16, C], BF16, tag="kpo", name="kpo")
            for lh in range(2):
                nc.scalar.dma_start(
                    out=kp2x[:, 8 * lh:8 * (lh + 1), :],
                    in_=knat_all[:, 4 * lh:4 * (lh + 1), :],
                    transpose=True)
                nc.vector.tensor_copy(kp_odd[:, 8 * lh:8 * (lh + 1), :],
                                      kp2x[D:C, 8 * lh:8 * (lh + 1), :])

            def kp_ap(p, c):
                t = 2 * c + p // 2
                if p % 2 == 0:
                    return kp2x[0:D, t, :]
                return kp_odd[:, t, :]

            # ---------------- P2: chunked causal linear attention ----------------
            # natural-layout O'^T per chunk: [128 t, 65] = A'_c^T-contracted
            # V^ plus prefix-state terms; division is a per-partition scalar
            # multiply straight out of PSUM (denominator = col 64).
            # attn_nat[lt]: [128 l, 256 f] (all 4 pairs of l-chunk lt).
            attn_nat = p2p.tile([C, 8, FPC], BF16, tag="anat",
                                name="attn_nat", bufs=1)
            tsbs, usbs, terms_all = [], [], []
            for p in range(4):
                vcols = slice(p * 65, p * 65 + 65)
                kcols = slice(p * D, (p + 1) * D)
                # per-chunk states T_c = K_c^T V^_c: 8 independent matmuls
                # into 2 PSUM tiles, one ACT copy each, then 4 tiny bf16 adds
                # for the prefix cover (no PSUM<->ACT ping-pong chain).
                tsb = p2p.tile([D, 8, 65], BF16, tag="ts", bufs=4,
                               name=f"ts_{p}")
                for half in range(2):
                    ups = psp.tile([D, 4, 65], F32, tag="ps",
                                   name=f"ups{p}_{half}")
                    for c4 in range(4):
                        c = 4 * half + c4
                        nc.tensor.matmul(ups[:, c4, :], knat_ap(c, kcols),
                                         vnat[c][:, vcols],
                                         start=True, stop=True)
                    nc.scalar.copy(tsb[:, 4 * half:4 * half + 4, :], ups[:])
                usb = p2p.tile([D, 4, 65], BF16, tag="us", bufs=4,
                               name=f"us_{p}")
                # usb: 0=u01, 1=t23, 2=u03, 3=u45
                nc.vector.tensor_add(usb[:, 0, :], tsb[:, 0, :], tsb[:, 1, :])
                nc.vector.tensor_add(usb[:, 1, :], tsb[:, 2, :], tsb[:, 3, :])
                nc.vector.tensor_add(usb[:, 2, :], usb[:, 0, :], usb[:, 1, :])
                nc.vector.tensor_add(usb[:, 3, :], tsb[:, 4, :], tsb[:, 5, :])
                t0, t2, t4, t6 = (tsb[:, 0, :], tsb[:, 2, :],
                                  tsb[:, 4, :], tsb[:, 6, :])
                u01, t23, u03, u45 = (usb[:, 0, :], usb[:, 1, :],
                                      usb[:, 2, :], usb[:, 3, :])
                # minimal prefix cover of T_0..T_{c-1} for each chunk c
                terms_all.append({
                    0: [], 1: [t0], 2: [u01], 3: [u01, t2],
                    4: [u03], 5: [u03, t4], 6: [u03, u45],
                    7: [u03, u45, t6],
                })
            NB_CH = 4
            # group-major: all pairs' chunks 0-3 finish first, so the first
            # attn transpose + a2a staging overlap the chunk 4-7 groups
            for cp2 in range(NCH // NB_CH):
                cI0 = NB_CH * cp2
                for p in range(4):
                    vcols = slice(p * 65, p * 65 + 65)
                    terms = terms_all[p]
                    # A'[s,t] for NB_CH chunks side by side (causal mask s<=t)
                    aps = psp.tile([C, NB_CH * C], F32, tag="ps")
                    for h in range(NB_CH):
                        cs = slice((cI0 + h) * C, (cI0 + h + 1) * C)
                        nc.tensor.matmul(aps[:, h * C:(h + 1) * C],
                                         kp_ap(p, cI0 + h), qp_t[p][:, cs],
                                         start=True, stop=True)
                    asb = p2p.tile([C, NB_CH * C], BF16, tag="asb")
                    nc.vector.tensor_mul(asb[:], aps[:], mask2[:])
                    # O'^T (128t, 65) = A'_c^T-contraction of V^ + Q'^T S
                    ops = psp.tile([C, NB_CH, 65], F32, tag="ps")
                    for h in range(NB_CH):
                        cI = cI0 + h
                        cs = slice(cI * C, (cI + 1) * C)
                        tl = terms[cI]
                        nc.tensor.matmul(ops[:, h, :], asb[:, h * C:(h + 1) * C],
                                         vnat[cI][:, vcols],
                                         start=True, stop=not tl)
                        for i2, tt in enumerate(tl):
                            nc.tensor.matmul(ops[:, h, :], qp_t[p][:, cs],
                                             tt,
                                             start=False, stop=(i2 == len(tl) - 1))
                    # division: denom = q.z is strictly positive and >=O(10)
                    # at these input scales; the reference's +1e-6 contributes
                    # ~1e-8 relative, far below bf16 precision, so skip it.
                    rcp = p2p.tile([C, NB_CH], F32, tag="rcp", bufs=4)
                    nc.vector.reciprocal(rcp[:], ops[:, :, 64])
                    for h in range(NB_CH):
                        dst = attn_nat[:, cI0 + h, p * D:(p + 1) * D]
                        if h % 2 == 0:
                            nc.vector.tensor_scalar(
                                dst, ops[:, h, 0:64], rcp[:, h:h + 1], None,
                                op0=mybir.AluOpType.mult)
                        else:
                            nc.scalar.activation(
                                dst, ops[:, h, 0:64],
                                mybir.ActivationFunctionType.Copy,
                                scale=rcp[:, h:h + 1])

            # ---------------- attn natural -> feature-major (DMA transpose) --
            # att3[:, t, :] = transpose of attn_nat block t = 2*lt + h
            # (h = pair-half), via two batched block-transposes
            att3 = attnp.tile([C, 16, C], BF16, tag="att3")
            for lh in range(4):
                e = nc.sync if lh % 2 == 0 else nc.scalar
                e.dma_start(
                    out=att3[:, 4 * lh:4 * (lh + 1), :],
                    in_=attn_nat[:, 2 * lh:2 * (lh + 1), :],
                    transpose=True)

            # ---------------- A2A: redistribute attention outputs ----------------
            # single merged collective: buffer row r = j*256 + h*128 + f
            # (j = dest l-chunk, h = pair-half, f = feature-in-half); shard j
            # is the contiguous 256-row block j.
            a2a_in = dram.tile([N_CORES, 2 * C, C], BF16, tag="a2ain")
            a2a_out = dram.tile([N_CORES, 2 * C, C], BF16, tag="a2aout")
            # stage in quarters (both pair-halves of 2 l-chunks at once)
            for lq in range(4):
                e = nc.sync if lq % 2 == 0 else nc.scalar
                e.dma_start(
                    out=a2a_in[lq * 2:(lq + 1) * 2, :, :]
                        .rearrange("j (h f) l -> f (j h) l", h=2),
                    in_=att3[:, 4 * lq:4 * (lq + 1), :],
                )
            # 2D APs: the cost model prices a collective by its non-leading
            # dims, so present the buffer as [2048, 128]
            nc.gpsimd.collective_compute(
                "AllToAll",
                mybir.AluOpType.bypass,
                replica_groups=[list(range(N_CORES))],
                ins=[a2a_in[:].rearrange("j f l -> (j f) l")],
                outs=[a2a_out[:].rearrange("j f l -> (j f) l")],
            )

            # ---------------- woT load (single big cast-DMA) ----------------
            wo_all = big_load(woTp, woT, 8, E, "wo_all", halves=2)
            wo_sb = [wo_all[:, k, :] for k in range(8)]

            # ---------------- P3: output projection ----------------
            # a2a_out rows for batch n are (b h f)-ordered: block s = 2b+h
            # matches wo_sb[s] (feature rows [128s, 128s+128)). Two DMAs per
            # batch so the kk-accumulation can start on the first half.
            ath = {n: atTp.tile([C, 8, C], BF16, tag=f"ath{n}", name=f"ath{n}")
                   for n in range(NB)}
            for hv in range(2):
                for n in range(NB):
                    e = nc.sync if n == 0 else nc.scalar
                    e.dma_start(
                        out=ath[n][:, hv * 4:(hv + 1) * 4, :],
                        in_=a2a_out[4 * n + 2 * hv:4 * n + 2 * hv + 2, :, :]
                            .rearrange("b (h f) l -> f (b h) l", f=C))
            ps3 = {}
            for n in range(NB):
                for nbk in range(2):
                    ps3[(n, nbk)] = psp.tile([C, 512], F32, tag="ps",
                                             name=f"ps3_{n}_{nbk}")
            # hv-outer: all four PSUM groups start on the first ath arrivals
            for hv in range(2):
                for n in range(NB):
                    for nbk in range(2):
                        for s4 in range(4):
                            kk = 4 * hv + s4
                            nc.tensor.matmul(
                                ps3[(n, nbk)][:],
                                ath[n][:, kk, :],
                                wo_sb[kk][:, nbk * 512:(nbk + 1) * 512],
                                start=(hv == 0 and s4 == 0),
                                stop=(hv == 1 and s4 == 3))
            for n in range(NB):
                osb = outp.tile([C, E], F32, tag=f"osb{n}", name=f"osb{n}")
                for nbk in range(2):
                    cs = slice(nbk * 512, (nbk + 1) * 512)
                    if with_bias:
                        nc.vector.tensor_add(osb[:, cs], ps3[(n, nbk)][:],
                                             bo_bc[:, cs])
                    elif (2 * n + nbk) % 2 == 0:
                        nc.scalar.copy(osb[:, cs], ps3[(n, nbk)][:])
                    else:
                        nc.vector.tensor_copy(osb[:, cs], ps3[(n, nbk)][:])
                    e = nc.sync if n == 0 else nc.scalar
                    e.dma_start(out=out_d[n][:, cs], in_=osb[:, cs])
            nc.sync.dma_start(out=tok_o[:], in_=tok_i[:])

    _split_waits(nc)
    return nc


def _run_pjrt_timed(nc, in_maps, time_iters=0):
    """Replicates bass2jax.run_bass_via_pjrt's multi-core path, but keeps
    inputs device-resident and (optionally) times repeated executions.
    Returns (results, best_exec_seconds_or_None)."""
    import time as _time
    import jax
    from jax.sharding import Mesh, PartitionSpec, NamedSharding
    from jax.experimental.shard_map import shard_map
    from concourse import bass2jax, mybir as mb

    bass2jax.install_neuronx_cc_hook()
    n_cores = len(in_maps)
    partition_name = nc.partition_id_tensor.name if nc.partition_id_tensor else None

    in_names, out_names, out_avals, zero_outs = [], [], [], []
    for alloc in nc.m.functions[0].allocations:
        if not isinstance(alloc, mb.MemoryLocationSet):
            continue
        name = alloc.memorylocations[0].name
        if alloc.kind == "ExternalInput":
            if name != partition_name:
                in_names.append(name)
        elif alloc.kind == "ExternalOutput":
            out_names.append(name)
            shape = tuple(alloc.tensor_shape)
            dtype = mb.dt.np(alloc.dtype)
            out_avals.append(jax.core.ShapedArray(shape, dtype))
            zero_outs.append(np.zeros(shape, dtype))
    n_params = len(in_names)
    in_names.extend(out_names)
    if partition_name is not None:
        in_names.append(partition_name)

    chain = int(__import__("os").environ.get("TRN_KERNEL_CHAIN", "1"))
    tok_in_idx = in_names.index("tok") if "tok" in in_names else None
    tok_out_idx = out_names.index("tok_out") if "tok_out" in out_names else None

    def _body(*args):
        operands = list(args)
        pid = bass2jax.partition_id_tensor() if partition_name is not None else None
        outs = None
        for _ in range(chain):
            ops = list(operands)
            if outs is not None and tok_in_idx is not None:
                ops[tok_in_idx] = outs[tok_out_idx]  # serialize iterations
            if pid is not None:
                ops.append(pid)
            outs = bass2jax._bass_exec_p.bind(
                *ops,
                out_avals=tuple(out_avals),
                in_names=tuple(in_names),
                out_names=tuple(out_names),
                lowering_input_output_aliases=(),
                sim_require_finite=True,
                sim_require_nnan=True,
                nc=nc,
            )
        return tuple(outs)

    devices = jax.devices()[:n_cores]
    mesh = Mesh(np.asarray(devices), ("core",))
    in_specs = (PartitionSpec("core"),) * (n_params + len(out_names))
    out_specs = (PartitionSpec("core"),) * len(out_names)
    sharded = jax.jit(
        shard_map(_body, mesh=mesh, in_specs=in_specs, out_specs=out_specs,
                  check_rep=False),
        keep_unused=True,
    )
    per_core = [[np.asarray(m[name]) for name in in_names[:n_params]]
                for m in in_maps]
    concat_in = [np.concatenate([per_core[c][i] for c in range(n_cores)], axis=0)
                 for i in range(n_params)]
    concat_zeros = [np.zeros((n_cores * z.shape[0], *z.shape[1:]), z.dtype)
                    for z in zero_outs]
    shd = NamedSharding(mesh, PartitionSpec("core"))
    dev_in = [jax.device_put(a, shd) for a in concat_in + concat_zeros]

    out_arrs = sharded(*dev_in)
    jax.block_until_ready(out_arrs)
    best = None
    for _ in range(time_iters):
        t0 = _time.perf_counter()
        out_arrs2 = sharded(*dev_in)
        jax.block_until_ready(out_arrs2)
        dt = _time.perf_counter() - t0
        best = dt if best is None or dt < best else best
    results = [
        {name: np.asarray(out_arrs[i]).reshape(n_cores, *out_avals[i].shape)[c]
         for i, name in enumerate(out_names)}
        for c in range(n_cores)
    ]
    return results, best


def kernel(**inputs):
    global LAST_RESULT
    import os

    query = np.asarray(inputs["query"], np.float32)
    key = np.asarray(inputs["key"], np.float32)
    value = np.asarray(inputs["value"], np.float32)
    Wq = np.asarray(inputs["Wq"], np.float32)
    Wk = np.asarray(inputs["Wk"], np.float32)
    Wv = np.asarray(inputs["Wv"], np.float32)
    Wo = np.asarray(inputs["Wo"], np.float32)
    bq = np.asarray(inputs["bq"], np.float32)
    bk = np.asarray(inputs["bk"], np.float32)
    bv = np.asarray(inputs["bv"], np.float32)
    bo = np.asarray(inputs["bo"], np.float32)

    with_bias = any(np.any(b) for b in (bq, bk, bv, bo))
    nc = _build(with_bias)

    from ml_dtypes import bfloat16 as _bf16

    def _b(x):
        return np.ascontiguousarray(x).astype(_bf16)

    woT_full = _b(Wo.T)
    in_maps = []
    for c in range(N_CORES):
        a, b = c // 4, c % 4
        F = slice(FPC * b, FPC * (b + 1))
        m = {
            "xqT": _b(query[:, a, :].T),
            "xkT": _b(key[:, a, :].T),
            "xvT": _b(value[:, a, :].T),
            "wqT": _b(Wq[F, :].T),
            "wkT": _b(Wk[F, :].T),
            "wvT": _b(Wv[F, :].T),
            "woT": woT_full,
            "tok": np.zeros((1, 1), np.float32),
        }
        if with_bias:
            m["bq"] = np.ascontiguousarray(bq[F].reshape(FPC, 1))
            m["bk"] = np.ascontiguousarray(bk[F].reshape(FPC, 1))
            m["bv"] = np.ascontiguousarray(bv[F].reshape(FPC, 1))
            m["bo"] = np.ascontiguousarray(bo.reshape(1, E))
        in_maps.append(m)

    time_iters = int(os.environ.get("TRN_KERNEL_TIME_ITERS", "0"))
    results, best = _run_pjrt_timed(nc, in_maps, time_iters=time_iters)
    LAST_RESULT = {"results": results, "best_exec_s": best}

    out = np.empty((L, NB, E), np.float32)
    for c in range(N_CORES):
        o = results[c]["out"]  # (NB, C, E): my l-chunk rows for both batches
        for n in range(NB):
            out[c * C:(c + 1) * C, n, :] = o[n]
    return out

